# revision 5
# baseline (speedup 1.0000x reference)
"""Trainium2 Bass kernel for nn_CrossAttention (single-query cross attention).

Reference computation (B=4, C=64, H=W=128, heads h=64, dim_head d=64,
inner=4096, HW=16384):
    x[b, j, c]   = fimg[b, c, j]                       (j indexes H*W)
    q[b, h, d]   = sum_e fpsf[b, e] Wq[h*64+d, e]
    k[b, j, h, d]= sum_c x[b, j, c] Wk[h*64+d, c]
    out[b, h, j] = scale * sum_d q[b,h,d] k[b,j,h,d]

Because there is a single query per (batch, head), the attention collapses:
    W2[b, h, c]  = scale * sum_d q[b,h,d] Wk[h*64+d, c]      (tiny)
    out[b, h, j] = sum_c W2[b,h,c] fimg[b, c, j]
a 64x FLOP reduction vs materializing k.

Sharding: the j (H*W = 16384) axis is split across the 8 cores (2048 each).
Every core redundantly computes W2 (it needs all heads for its output).

Profile-driven history:
  v1 29.0us  v2 26.6us (DMA issues 11->6, weights-before-fimg, bf16 out)
  v4: dense Wk (in-DMA 2.5MB -> 2.0MB/core), PE warm-up matmuls (HAM
      cold clock halved every matmul before), per-pair double-LDW for
      step B keeps the 128-col stationary (compiler FWL).

Device layouts (prepared host-side; host does LAYOUT only, no math):
  WqF    [64, 4100] bf16: cols 0:4 = fpsf.T, cols 4: = Wq.T  (kept at
                         base partition 0: bf16 matmuls with operands at
                         partition offset 64 crash TRN2)
  Wk_nat [64, 4096] bf16: Wk_nat[d, 64h+c] = Wk[64h+d, c]
  fimg_s [128, 4096] bf16: rows b%2*64+c, cols 2048*(b//2) + local j
  out    [128, 4096] bf16: rows b%2*64+h, cols 2048*(b//2) + local j

Device compute per core:
  warm: memset + 8 junk 512-col matmuls fill the HAM activity window so
     the PE clock is at 2.4GHz (not the cold 1.2) by the time real
     matmuls run.
  A: 32 matmuls  q2T chunk [128, 4] = WqT_chunk.T @ fpsfT
     -> q2T psum [128, 128] with rows d+64*(h%2), cols 4*(h//2)+b
  copy: q2T psum halves -> SBUF bf16 q2e/q2o [64, 128] (scale folded)
  B: 64 matmuls, head-pair p loads Wk_nat[:, 128p:128p+128] (128-col
     stationary -> FWL) twice: rhs q2e[:, 4p:4p+4] -> w2a[:, 4p:4p+4]
     (rows 0:64 valid), rhs q2o -> w2b (rows 64:128 valid).
  Assembly: per batch-pair q, block-diag lhsT bd_q [128, 128] (bf16):
     bd_q[64*half + c, 64*half + h] = W2[2q+half, h, c]
  Big: 8 matmuls [128, 512] = bd_q.T @ fimg cols; psum -> bf16 SBUF
     staging [128, 2048] per q (vector/scalar alternate); out DMA per q.
"""

import sys
import types

import numpy as np
import ml_dtypes

# antenv.axon_hooks is absent in this image; bass_utils imports it when
# tracing. Register a minimal stand-in before importing concourse.
if "antenv.axon_hooks" not in sys.modules:
    try:
        import antenv  # noqa: F401

        _hooks = types.ModuleType("antenv.axon_hooks")
        _hooks._hook = None

        def _set_hook(h):
            _hooks._hook = h

        _hooks.set_axon_ntff_profile_hook = _set_hook
        _hooks.get_axon_ntff_profile_hook = lambda: _hooks._hook
        sys.modules["antenv.axon_hooks"] = _hooks
        try:
            from trn_agent_boot.trn_boot import _ntff_profile_via_ctypes

            _set_hook(_ntff_profile_via_ctypes("/opt/axon/libaxon_pjrt.so"))
        except Exception:
            pass
    except ImportError:
        pass

import concourse.bass as bass  # noqa: E402
import concourse.mybir as mybir  # noqa: E402
import concourse.tile as tile  # noqa: E402
from concourse import bacc  # noqa: E402
from concourse.bass_utils import run_bass_kernel_spmd  # noqa: E402

N_CORES = 8
B, C, H, W = 4, 64, 128, 128
HEADS, DIM_HEAD = 64, 64
HW = H * W
JS = HW // N_CORES  # 2048 j-positions per core
SCALE = DIM_HEAD ** -0.5
F32 = mybir.dt.float32
BF16 = mybir.dt.bfloat16
NPBF16 = ml_dtypes.bfloat16

_compiled = None  # cache (nc) across calls


def _build():
    nc = bacc.Bacc("TRN2", target_bir_lowering=False, debug=False,
                   num_devices=N_CORES)

    fimg_d = nc.dram_tensor("fimg_s", [128, 2 * JS], BF16, kind="ExternalInput")
    wqf_d = nc.dram_tensor("WqF", [64, 4100], BF16, kind="ExternalInput")
    wk_d = nc.dram_tensor("Wk_nat", [64, 4096], BF16, kind="ExternalInput")
    out_d = nc.dram_tensor("out", [128, 2 * JS], BF16, kind="ExternalOutput")

    with tile.TileContext(nc) as tc:
        with (
            tc.tile_pool(name="weights", bufs=1) as wpool,
            tc.tile_pool(name="img", bufs=1) as ipool,
            tc.tile_pool(name="small_ps", bufs=1, space="PSUM") as spsum,
            tc.tile_pool(name="big_ps", bufs=5, space="PSUM") as bpsum,
            tc.tile_pool(name="ostage", bufs=2) as opool,
        ):
            # Input DMAs in priority order on the Sync HWDGE ring (FIFO
            # per issuing engine): wqf gates step A, wk gates step B,
            # fimg halves gate the big matmuls at the end.
            wqf = wpool.tile([64, 4100], BF16, tag="wqf")
            nc.sync.dma_start(wqf[:], wqf_d.ap()[:])
            fpsfT = wqf[:, 0:4]
            wqT = wqf[:, 4:4100]
            wk = wpool.tile([64, 4096], BF16, tag="wk")
            nc.sync.dma_start(wk[:], wk_d.ap()[:])
            imgs = ipool.tile([128, 2 * JS], BF16, tag="img")
            nc.sync.dma_start(imgs[:, 0:JS], fimg_d.ap()[:, 0:JS])
            nc.sync.dma_start(imgs[:, JS:2 * JS], fimg_d.ap()[:, JS:2 * JS])

            # Warm-up: ~3.4us of junk matmuls flips the PE HAM clock gate
            # to 8/8 (2.4GHz) before any real matmul issues; the PE is
            # otherwise idle while the weight DMAs land.
            warm = wpool.tile([128, 640], BF16, tag="warm")
            nc.vector.memset(warm[:], 0.0)
            for _ in range(8):
                wps = bpsum.tile([128, 512], F32, tag="mm_ps")
                nc.tensor.matmul(wps[:], warm[:, 0:128], warm[:, 128:640],
                                 start=True, stop=True)

            # A: q2T[d + 64*(h%2), 4*(h//2)+b] = q[b, h, d] (unscaled)
            q2T_ps = spsum.tile([128, 128], F32, tag="q2T_ps")
            for p in range(32):
                nc.tensor.matmul(
                    q2T_ps[:, 4 * p:4 * p + 4],
                    wqT[:, 128 * p:128 * p + 128],
                    fpsfT,
                    start=True, stop=True,
                )
            # Scale folded into the PSUM->SBUF copies; split halves so
            # both B operands sit at base partition 0.
            q2e = wpool.tile([64, 128], BF16, tag="q2e")
            q2o = wpool.tile([64, 128], BF16, tag="q2o")
            nc.vector.tensor_scalar_mul(q2e[:], q2T_ps[0:64, :], SCALE)
            nc.vector.tensor_scalar_mul(q2o[:], q2T_ps[64:128, :], SCALE)

            # B: pair p stationary = Wk_nat[:, 128p:128p+128] (cols 0:64 =
            # head 2p, 64:128 = head 2p+1). Two matmuls reuse it:
            #   w2a[c, 4p+b]      = W2[b, 2p, c]      (rows 64: garbage)
            #   w2b[64+c, 4p+b]   = W2[b, 2p+1, c]    (rows <64: garbage)
            w2a = spsum.tile([128, 128], F32, tag="w2a")
            w2b = spsum.tile([128, 128], F32, tag="w2b")
            for p in range(32):
                lhsT = wk[:, 128 * p:128 * p + 128]
                nc.tensor.matmul(w2a[:, 4 * p:4 * p + 4], lhsT,
                                 q2e[:, 4 * p:4 * p + 4], start=True, stop=True)
                nc.tensor.matmul(w2b[:, 4 * p:4 * p + 4], lhsT,
                                 q2o[:, 4 * p:4 * p + 4], start=True, stop=True)

            # Assembly: bd_q[64*half + c, 64*half + h] = W2[2q+half, h, c]
            bds = []
            for q in range(2):
                bd = wpool.tile([128, 128], BF16, tag=f"bd{q}")
                nc.vector.memset(bd[:], 0.0)
                for half in range(2):
                    b = 2 * q + half
                    for parity in range(2):
                        dst = bd[64 * half:64 * half + 64,
                                 64 * half + parity:64 * half + 64:2]
                        src = (w2a if parity == 0 else w2b)[
                            64 * parity:64 * parity + 64, b:128:4]
                        nc.vector.tensor_copy(dst, src)
                bds.append(bd)

            # Big: out rows pair q = bd_q.T @ img_q, in 512-col chunks.
            # PSUM -> bf16 staging (vector/scalar alternate), one output
            # DMA per q on the Sync ring (idle after the input issues).
            for q in range(2):
                ot = opool.tile([128, JS], BF16, tag=f"ot{q}")
                for k in range(4):
                    ps = bpsum.tile([128, 512], F32, tag="mm_ps")
                    nc.tensor.matmul(
                        ps[:], bds[q][:],
                        imgs[:, JS * q + 512 * k:JS * q + 512 * k + 512],
                        start=True, stop=True,
                    )
                    dst = ot[:, 512 * k:512 * k + 512]
                    if k % 2 == 0:
                        nc.vector.tensor_copy(dst, ps[:])
                    else:
                        nc.scalar.copy(dst, ps[:])
                nc.sync.dma_start(
                    out_d.ap()[:, JS * q:JS * (q + 1)], ot[:])

    nc.compile()
    return nc


def _prep_inputs(fpsf, fimg, Wq, Wk):
    fpsf = np.ascontiguousarray(fpsf, dtype=np.float32)
    fimg = np.ascontiguousarray(fimg, dtype=np.float32)
    Wq = np.ascontiguousarray(Wq, dtype=np.float32)
    Wk = np.ascontiguousarray(Wk, dtype=np.float32)

    WqF = np.empty((64, 4100), NPBF16)
    WqF[:, 0:4] = fpsf.T.astype(NPBF16)
    WqF[:, 4:4100] = Wq.T.astype(NPBF16)

    # Wk_nat[d, 64h+c] = Wk[64h+d, c]
    Wk_nat = np.ascontiguousarray(
        Wk.reshape(64, 64, 64).transpose(1, 0, 2).reshape(64, 4096)
    ).astype(NPBF16)

    fimg_f = fimg.reshape(B, C, HW).astype(NPBF16)
    in_maps = []
    for i in range(N_CORES):
        sh = np.ascontiguousarray(
            fimg_f[:, :, JS * i:JS * (i + 1)]).reshape(2, 128, JS)
        sh = np.ascontiguousarray(
            sh.transpose(1, 0, 2).reshape(128, 2 * JS))
        in_maps.append({
            "fimg_s": sh,
            "WqF": WqF,
            "Wk_nat": Wk_nat,
        })
    return in_maps


def kernel(fpsf, fimg, Wq, Wk):
    global _compiled
    if _compiled is None:
        _compiled = _build()
    nc = _compiled

    in_maps = _prep_inputs(fpsf, fimg, Wq, Wk)
    res = run_bass_kernel_spmd(nc, in_maps, core_ids=list(range(N_CORES)))

    out = np.empty((B, HEADS, HW), dtype=np.float32)
    for i in range(N_CORES):
        o = res.results[i]["out"]  # [128, 2*JS] bf16
        o = o.reshape(128, 2, JS).transpose(1, 0, 2).reshape(B, HEADS, JS)
        out[:, :, JS * i:JS * (i + 1)] = o.astype(np.float32)
    return out.reshape(B, C, H, W)


if __name__ == "__main__":
    rng = np.random.default_rng(0)
    ins = {
        "fpsf": rng.standard_normal((B, C), dtype=np.float32),
        "fimg": rng.standard_normal((B, C, H, W), dtype=np.float32),
        "Wq": (rng.standard_normal((4096, C), dtype=np.float32) * 0.05),
        "Wk": (rng.standard_normal((4096, C), dtype=np.float32) * 0.05),
    }
    out = kernel(**ins)
    print("out", out.shape, out.dtype, float(np.abs(out).max()))


# revision 8
# speedup vs baseline: 1.0172x; 1.0172x over previous
"""Trainium2 Bass kernel for nn_CrossAttention (single-query cross attention).

Reference computation (B=4, C=64, H=W=128, heads h=64, dim_head d=64,
inner=4096, HW=16384):
    x[b, j, c]   = fimg[b, c, j]                       (j indexes H*W)
    q[b, h, d]   = sum_e fpsf[b, e] Wq[h*64+d, e]
    k[b, j, h, d]= sum_c x[b, j, c] Wk[h*64+d, c]
    out[b, h, j] = scale * sum_d q[b,h,d] k[b,j,h,d]

Because there is a single query per (batch, head), the attention collapses:
    W2[b, h, c]  = scale * sum_d q[b,h,d] Wk[h*64+d, c]      (tiny)
    out[b, h, j] = sum_c W2[b,h,c] fimg[b, c, j]
a 64x FLOP reduction vs materializing k.

Sharding: the j (H*W = 16384) axis is split across the 8 cores (2048 each).
Every core redundantly computes W2 (it needs all heads for its output).

Profile-driven history:
  v1 29.0us  v2 26.6us (DMA issues 11->6, weights-before-fimg, bf16 out)
  v4: dense Wk (in-DMA 2.5MB -> 2.0MB/core), PE warm-up matmuls (HAM
      cold clock halved every matmul before), per-pair double-LDW for
      step B keeps the 128-col stationary (compiler FWL).

Device layouts (prepared host-side; host does LAYOUT only, no math):
  WqF    [64, 4100] bf16: cols 0:4 = fpsf.T, cols 4: = Wq.T  (kept at
                         base partition 0: bf16 matmuls with operands at
                         partition offset 64 crash TRN2)
  Wk_nat [64, 4096] bf16: Wk_nat[d, 64h+c] = Wk[64h+d, c]
  fimg_s [128, 4096] bf16: rows b%2*64+c, cols 2048*(b//2) + local j
  out    [128, 4096] bf16: rows b%2*64+h, cols 2048*(b//2) + local j

Device compute per core:
  warm: memset + 8 junk 512-col matmuls fill the HAM activity window so
     the PE clock is at 2.4GHz (not the cold 1.2) by the time real
     matmuls run.
  A: 32 matmuls  q2T chunk [128, 4] = WqT_chunk.T @ fpsfT
     -> q2T psum [128, 128] with rows d+64*(h%2), cols 4*(h//2)+b
  copy: q2T psum halves -> SBUF bf16 q2e/q2o [64, 128] (scale folded)
  B: 64 matmuls, head-pair p loads Wk_nat[:, 128p:128p+128] (128-col
     stationary -> FWL) twice: rhs q2e[:, 4p:4p+4] -> w2a[:, 4p:4p+4]
     (rows 0:64 valid), rhs q2o -> w2b (rows 64:128 valid).
  Assembly: per batch-pair q, block-diag lhsT bd_q [128, 128] (bf16):
     bd_q[64*half + c, 64*half + h] = W2[2q+half, h, c]
  Big: 8 matmuls [128, 512] = bd_q.T @ fimg cols; psum -> bf16 SBUF
     staging [128, 2048] per q (vector/scalar alternate); out DMA per q.
"""

import sys
import types

import numpy as np
import ml_dtypes

# antenv.axon_hooks is absent in this image; bass_utils imports it when
# tracing. Register a minimal stand-in before importing concourse.
if "antenv.axon_hooks" not in sys.modules:
    try:
        import antenv  # noqa: F401

        _hooks = types.ModuleType("antenv.axon_hooks")
        _hooks._hook = None

        def _set_hook(h):
            _hooks._hook = h

        _hooks.set_axon_ntff_profile_hook = _set_hook
        _hooks.get_axon_ntff_profile_hook = lambda: _hooks._hook
        sys.modules["antenv.axon_hooks"] = _hooks
        try:
            from trn_agent_boot.trn_boot import _ntff_profile_via_ctypes

            _set_hook(_ntff_profile_via_ctypes("/opt/axon/libaxon_pjrt.so"))
        except Exception:
            pass
    except ImportError:
        pass

import concourse.bass as bass  # noqa: E402
import concourse.mybir as mybir  # noqa: E402
import concourse.tile as tile  # noqa: E402
from concourse import bacc  # noqa: E402
from concourse.bass_utils import run_bass_kernel_spmd  # noqa: E402

N_CORES = 8
B, C, H, W = 4, 64, 128, 128
HEADS, DIM_HEAD = 64, 64
HW = H * W
JS = HW // N_CORES  # 2048 j-positions per core
SCALE = DIM_HEAD ** -0.5
F32 = mybir.dt.float32
BF16 = mybir.dt.bfloat16
NPBF16 = ml_dtypes.bfloat16

_compiled = None  # cache (nc) across calls


def _build():
    nc = bacc.Bacc("TRN2", target_bir_lowering=False, debug=False,
                   num_devices=N_CORES)

    fimg_d = nc.dram_tensor("fimg_s", [128, 2 * JS], BF16, kind="ExternalInput")
    wqf_d = nc.dram_tensor("WqF", [64, 4100], BF16, kind="ExternalInput")
    wk_d = nc.dram_tensor("Wk_nat", [64, 4096], BF16, kind="ExternalInput")
    out_d = nc.dram_tensor("out", [128, 2 * JS], BF16, kind="ExternalOutput")

    with tile.TileContext(nc) as tc:
        with (
            tc.tile_pool(name="weights", bufs=1) as wpool,
            tc.tile_pool(name="img", bufs=1) as ipool,
            tc.tile_pool(name="small_ps", bufs=1, space="PSUM") as spsum,
            tc.tile_pool(name="big_ps", bufs=5, space="PSUM") as bpsum,
            tc.tile_pool(name="ostage", bufs=1) as opool,
        ):
            # Input DMAs in priority order on the Sync HWDGE ring (FIFO
            # per issuing engine): wqf gates step A, wk gates step B,
            # fimg halves gate the big matmuls at the end.
            wqf = wpool.tile([64, 4100], BF16, tag="wqf")
            nc.sync.dma_start(wqf[:, 0:2052], wqf_d.ap()[:, 0:2052])
            nc.sync.dma_start(wqf[:, 2052:4100], wqf_d.ap()[:, 2052:4100])
            fpsfT = wqf[:, 0:4]
            wqT = wqf[:, 4:4100]
            wk = wpool.tile([64, 4096], BF16, tag="wk")
            nc.sync.dma_start(wk[:, 0:2048], wk_d.ap()[:, 0:2048])
            nc.sync.dma_start(wk[:, 2048:4096], wk_d.ap()[:, 2048:4096])
            imgs = ipool.tile([128, 2 * JS], BF16, tag="img")
            for c in range(4):
                nc.sync.dma_start(imgs[:, 1024 * c:1024 * c + 1024],
                                  fimg_d.ap()[:, 1024 * c:1024 * c + 1024])

            # Warm-up: ~3.4us of junk matmuls flips the PE HAM clock gate
            # to 8/8 (2.4GHz) before any real matmul issues; the PE is
            # otherwise idle while the weight DMAs land.
            warm = wpool.tile([128, 640], BF16, tag="warm")
            nc.vector.memset(warm[:], 0.0)
            for _ in range(8):
                wps = bpsum.tile([128, 512], F32, tag="mm_ps")
                nc.tensor.matmul(wps[:], warm[:, 0:128], warm[:, 128:640],
                                 start=True, stop=True)

            # A: q2T[d + 64*(h%2), 4*(h//2)+b] = q[b, h, d] (unscaled)
            q2T_ps = spsum.tile([128, 128], F32, tag="q2T_ps")
            for p in range(32):
                nc.tensor.matmul(
                    q2T_ps[:, 4 * p:4 * p + 4],
                    wqT[:, 128 * p:128 * p + 128],
                    fpsfT,
                    start=True, stop=True,
                )
            # Scale folded into the PSUM->SBUF copies; split halves so
            # both B operands sit at base partition 0.
            q2e = wpool.tile([64, 128], BF16, tag="q2e")
            q2o = wpool.tile([64, 128], BF16, tag="q2o")
            nc.vector.tensor_scalar_mul(q2e[:], q2T_ps[0:64, :], SCALE)
            nc.vector.tensor_scalar_mul(q2o[:], q2T_ps[64:128, :], SCALE)

            # B: pair p stationary = Wk_nat[:, 128p:128p+128] (cols 0:64 =
            # head 2p, 64:128 = head 2p+1). Two matmuls reuse it:
            #   w2a[c, 4p+b]      = W2[b, 2p, c]      (rows 64: garbage)
            #   w2b[64+c, 4p+b]   = W2[b, 2p+1, c]    (rows <64: garbage)
            w2a = spsum.tile([128, 128], F32, tag="w2a")
            w2b = spsum.tile([128, 128], F32, tag="w2b")
            for p in range(32):
                lhsT = wk[:, 128 * p:128 * p + 128]
                nc.tensor.matmul(w2a[:, 4 * p:4 * p + 4], lhsT,
                                 q2e[:, 4 * p:4 * p + 4], start=True, stop=True)
                nc.tensor.matmul(w2b[:, 4 * p:4 * p + 4], lhsT,
                                 q2o[:, 4 * p:4 * p + 4], start=True, stop=True)

            # Assembly: bd_q[64*half + c, 64*half + h] = W2[2q+half, h, c]
            bds = []
            for q in range(2):
                bd = wpool.tile([128, 128], BF16, tag=f"bd{q}")
                nc.vector.memset(bd[:], 0.0)
                for half in range(2):
                    b = 2 * q + half
                    for parity in range(2):
                        dst = bd[64 * half:64 * half + 64,
                                 64 * half + parity:64 * half + 64:2]
                        src = (w2a if parity == 0 else w2b)[
                            64 * parity:64 * parity + 64, b:128:4]
                        nc.vector.tensor_copy(dst, src)
                bds.append(bd)

            # Bridge warm-up: keep the PE HAM window busy while the bd
            # assembly copies run on DVE, so the big matmuls stay at
            # 2.4GHz (measured: HAM re-throttles during B's ~50% duty).
            for _ in range(3):
                wps = bpsum.tile([128, 512], F32, tag="mm_ps")
                nc.tensor.matmul(wps[:], warm[:, 0:128], warm[:, 128:640],
                                 start=True, stop=True)

            # Big: 1024-col chunk c (gated on fimg chunk c): 2 matmuls of
            # 512, PSUM -> bf16 staging (vector + scalar in parallel),
            # then the chunk's output DMA on the Sync ring (idle by now).
            for cch in range(4):
                q = cch // 2
                ot = opool.tile([128, 1024], BF16, tag=f"ot{cch}")
                for k in range(2):
                    col = 1024 * cch + 512 * k
                    ps = bpsum.tile([128, 512], F32, tag="mm_ps")
                    nc.tensor.matmul(
                        ps[:], bds[q][:], imgs[:, col:col + 512],
                        start=True, stop=True,
                    )
                    dst = ot[:, 512 * k:512 * k + 512]
                    if k % 2 == 0:
                        nc.vector.tensor_copy(dst, ps[:])
                    else:
                        nc.scalar.copy(dst, ps[:])
                nc.sync.dma_start(
                    out_d.ap()[:, 1024 * cch:1024 * cch + 1024], ot[:])

    nc.compile()
    return nc


def _prep_inputs(fpsf, fimg, Wq, Wk):
    fpsf = np.ascontiguousarray(fpsf, dtype=np.float32)
    fimg = np.ascontiguousarray(fimg, dtype=np.float32)
    Wq = np.ascontiguousarray(Wq, dtype=np.float32)
    Wk = np.ascontiguousarray(Wk, dtype=np.float32)

    WqF = np.empty((64, 4100), NPBF16)
    WqF[:, 0:4] = fpsf.T.astype(NPBF16)
    WqF[:, 4:4100] = Wq.T.astype(NPBF16)

    # Wk_nat[d, 64h+c] = Wk[64h+d, c]
    Wk_nat = np.ascontiguousarray(
        Wk.reshape(64, 64, 64).transpose(1, 0, 2).reshape(64, 4096)
    ).astype(NPBF16)

    fimg_f = fimg.reshape(B, C, HW).astype(NPBF16)
    in_maps = []
    for i in range(N_CORES):
        sh = np.ascontiguousarray(
            fimg_f[:, :, JS * i:JS * (i + 1)]).reshape(2, 128, JS)
        sh = np.ascontiguousarray(
            sh.transpose(1, 0, 2).reshape(128, 2 * JS))
        in_maps.append({
            "fimg_s": sh,
            "WqF": WqF,
            "Wk_nat": Wk_nat,
        })
    return in_maps


def kernel(fpsf, fimg, Wq, Wk):
    global _compiled
    if _compiled is None:
        _compiled = _build()
    nc = _compiled

    in_maps = _prep_inputs(fpsf, fimg, Wq, Wk)
    res = run_bass_kernel_spmd(nc, in_maps, core_ids=list(range(N_CORES)))

    out = np.empty((B, HEADS, HW), dtype=np.float32)
    for i in range(N_CORES):
        o = res.results[i]["out"]  # [128, 2*JS] bf16
        o = o.reshape(128, 2, JS).transpose(1, 0, 2).reshape(B, HEADS, JS)
        out[:, :, JS * i:JS * (i + 1)] = o.astype(np.float32)
    return out.reshape(B, C, H, W)


if __name__ == "__main__":
    rng = np.random.default_rng(0)
    ins = {
        "fpsf": rng.standard_normal((B, C), dtype=np.float32),
        "fimg": rng.standard_normal((B, C, H, W), dtype=np.float32),
        "Wq": (rng.standard_normal((4096, C), dtype=np.float32) * 0.05),
        "Wk": (rng.standard_normal((4096, C), dtype=np.float32) * 0.05),
    }
    out = kernel(**ins)
    print("out", out.shape, out.dtype, float(np.abs(out).max()))
